# revision 18
# baseline (speedup 1.0000x reference)
"""Trainium2 Bass kernel for PhyloAttention (16-head causal ALiBi attention).

Sharding: 8 cores; core c handles batch c//4 and heads [ci, 4+ci, 8+ci, 12+ci]
(ci = c%4, slot-major). Each core computes QKV projection for its 4 heads on
its batch, causal attention with ALiBi folded into the softmax as a per-key
exponential factor (the per-query factor cancels in normalization), and a
row-sharded partial output projection. Host sums the 8 partials.

fast2 path (used for the standard alibi/tril/zero-qk-bias case): all matmul
operands bf16 (halves DMA, full PE rate at any tile width), V computed
directly token-major (no PE transposes), ALiBi block-sparsity caps derived
from actual exp decay (dcaps [2,3,6,16] per slot vs [4,10,16,16]), softmax
normalization via DVE reciprocal + gpsimd partition_broadcast, and a flat
schedule over global supertiles: the next supertile's QKV chains (crossing
rep boundaries, with parity-double-buffered q/k/v) are interleaved between
attention slots so the PE never drains at rep boundaries.

Notes:
- phylo_dists adds a per-batch constant to all logits -> softmax-invariant,
  mathematically a no-op; it is ignored.
- fast2 requires attn_mask == tril, alibi_bias == standard ALiBi, and zero
  q/k biases (all verified numerically on host). Otherwise falls back to the
  original fast/causal_dram/full_dram paths.
"""
import sys
sys.path.insert(0, '/opt/trn_rl_repo')
import numpy as np

B, L, D, H = 2, 2048, 1024, 16
HD = D // H
SCALE = HD ** -0.5
NCORES = 8
NB = L // 128          # 16 key blocks per batch
NS = L // 512          # 4 query supertiles per batch

_cache = {}


def _build(variant, dcaps, ncols, stab, repeat=1):
    """variant: 'fast' | 'causal_dram' | 'full_dram'"""
    import concourse.bacc as bacc
    import concourse.mybir as mybir
    from concourse.tile import TileContext

    F32 = mybir.dt.float32
    F32R = mybir.dt.float32r
    AF = mybir.ActivationFunctionType
    ALU = mybir.AluOpType

    use_dram_bias = variant in ("causal_dram", "full_dram")
    causal = variant in ("fast", "causal_dram")

    nc = bacc.Bacc("TRN2", target_bir_lowering=False, debug=False)

    xt = nc.dram_tensor("xt", [D, L], F32R, kind="ExternalInput")
    wq = nc.dram_tensor("wq", [128, 2048], F32R, kind="ExternalInput")
    wk = nc.dram_tensor("wk", [128, 2048], F32R, kind="ExternalInput")
    wv = nc.dram_tensor("wv", [128, 2048], F32R, kind="ExternalInput")
    bqv = nc.dram_tensor("bqv", [128, 2], F32, kind="ExternalInput")
    bkv = nc.dram_tensor("bkv", [128, 2], F32, kind="ExternalInput")
    btab = nc.dram_tensor("btab", [128, 80], F32, kind="ExternalInput")
    onesd = nc.dram_tensor("onesd", [1, 64], F32R, kind="ExternalInput")
    vones = nc.dram_tensor("vones", [128, 16], F32R, kind="ExternalInput")
    identd = nc.dram_tensor("identd", [128, 128], F32R, kind="ExternalInput")
    wout = nc.dram_tensor("wout", [128, 2048], F32R, kind="ExternalInput")
    if use_dram_bias:
        biast = nc.dram_tensor("biast", [4, L, L], F32R, kind="ExternalInput")
        stabd = nc.dram_tensor("stabd", [128, 1], F32, kind="ExternalInput")
    out = nc.dram_tensor("out", [L, D], F32, kind="ExternalOutput")

    with TileContext(nc) as tc:
        with tc.tile_pool(name="const", bufs=1) as cp:
            wq_sb = cp.tile([128, 2048], F32R, tag="wq")
            wk_sb = cp.tile([128, 2048], F32R, tag="wk")
            wv_sb = cp.tile([128, 2048], F32R, tag="wv")
            nc.sync.dma_start(out=wq_sb[:, :], in_=wq[:, :])
            nc.sync.dma_start(out=wk_sb[:, :], in_=wk[:, :])
            nc.sync.dma_start(out=wv_sb[:, :], in_=wv[:, :])
            bq_sb = cp.tile([128, 2], F32, tag="bq")
            bk_sb = cp.tile([128, 2], F32, tag="bk")
            nc.sync.dma_start(out=bq_sb[:, :], in_=bqv[:, :])
            nc.sync.dma_start(out=bk_sb[:, :], in_=bkv[:, :])
            btab_sb = cp.tile([128, 80], F32, tag="btab")
            nc.sync.dma_start(out=btab_sb[:, :], in_=btab[:, :])
            ones_sb = cp.tile([1, 64], F32R, tag="ones")
            nc.sync.dma_start(out=ones_sb[:, :], in_=onesd[:, :])
            ident_sb = cp.tile([128, 128], F32R, tag="ident")
            nc.sync.dma_start(out=ident_sb[:, :], in_=identd[:, :])
            wout_sb = cp.tile([128, 2048], F32R, tag="wout")
            nc.sync.dma_start(out=wout_sb[:, :], in_=wout[:, :])
            if use_dram_bias:
                stab_sb = cp.tile([128, 1], F32, tag="stab")
                nc.sync.dma_start(out=stab_sb[:, :], in_=stabd[:, :])

            q_sb = [[cp.tile([128, 512], F32R, tag=f"q{t}s{s}", name=f"q{t}s{s}")
                     for s in range(NS)] for t in range(2)]
            k_sb = [[cp.tile([128, 512], F32R, tag=f"k{t}s{s}", name=f"k{t}s{s}")
                     for s in range(NS)] for t in range(2)]
            v_sb = [[cp.tile([128, 4 * 130], F32R, tag=f"v{t}s{s}", name=f"v{t}s{s}")
                     for s in range(NS)] for t in range(2)]
            ao_sb = [[cp.tile([128, 512], F32R, tag=f"ao{t}s{s}", name=f"ao{t}s{s}")
                      for s in range(NS)] for t in range(2)]

            # ones columns of v_sb (col 64 and 129 of each 130-block)
            for t in range(2):
                for s in range(NS):
                    v3 = v_sb[t][s].rearrange("p (blk c) -> p blk c", c=130)
                    vo3 = vones.rearrange("p (blk c) -> p blk c", c=1)
                    nc.sync.dma_start(out=v3[:, :, 64:65], in_=vo3[:, 0:4, :])
                    nc.sync.dma_start(out=v3[:, :, 129:130], in_=vo3[:, 0:4, :])

            # ---------------- Phase 1: QKV projection ----------------
            for _rep in range(repeat):
              with tc.tile_pool(name="xtp", bufs=1) as xtp, \
                   tc.tile_pool(name="vtp", bufs=2) as vtp, \
                   tc.tile_pool(name="ps1", bufs=1, space="PSUM") as ps1, \
                   tc.tile_pool(name="ps1t", bufs=1, space="PSUM") as ps1t, \
                   tc.tile_pool(name="spp", bufs=3, space="PSUM") as spp, \
                   tc.tile_pool(name="opp", bufs=2, space="PSUM") as opp, \
                   tc.tile_pool(name="lbp", bufs=1, space="PSUM") as lbp, \
                   tc.tile_pool(name="ptp", bufs=6) as ptp, \
                   tc.tile_pool(name="btp", bufs=3) as btp, \
                   tc.tile_pool(name="lrp", bufs=2) as lrp, \
                   tc.tile_pool(name="lvp", bufs=2) as lvp, \
                   tc.tile_pool(name="osp", bufs=3) as osp:
                  xts = []
                  for d in range(8):
                      xt_t = xtp.tile([128, L], F32R, tag=f"xt{d}", name=f"xt{d}")
                      nc.sync.dma_start(out=xt_t[:, :],
                                        in_=xt[d * 128:(d + 1) * 128, :])
                      xts.append(xt_t)
                  def emit_qkv(s):
                      for t in range(2):
                          pq = ps1.tile([128, 512], F32, tag="pq", name="pq")
                          pk = ps1.tile([128, 512], F32, tag="pk", name="pk")
                          pv = lbp.tile([128, 512], F32, tag="lbcop", name="pv")
                          for d in range(8):
                              xs = xts[d][:, s * 512:(s + 1) * 512]
                              st, sp_ = (d == 0), (d == 7)
                              c0 = d * 256 + t * 128
                              nc.tensor.matmul(pq[:, :], wq_sb[:, c0:c0 + 128],
                                               xs, start=st, stop=sp_)
                              nc.tensor.matmul(pk[:, :], wk_sb[:, c0:c0 + 128],
                                               xs, start=st, stop=sp_)
                              nc.tensor.matmul(pv[:, :], wv_sb[:, c0:c0 + 128],
                                               xs, start=st, stop=sp_)
                          nc.scalar.activation(q_sb[t][s][:, :],
                                               pq[:, :], AF.Identity,
                                               bias=bq_sb[:, t:t + 1])
                          nc.scalar.activation(k_sb[t][s][:, :],
                                               pk[:, :], AF.Identity,
                                               bias=bk_sb[:, t:t + 1])
                          # v: [f, tok] -> transpose 128x128 blocks -> [tok, f]
                          vt = vtp.tile([128, 512], F32R, tag="vt")
                          nc.vector.tensor_copy(vt[:, :], pv[:, :])
                          for blk in range(4):
                              ptv = lbp.tile([128, 128], F32R, tag="lbcop", name="ptv")
                              nc.tensor.transpose(ptv[:, :],
                                                  vt[:, blk * 128:(blk + 1) * 128],
                                                  ident_sb[:, :])
                              for hh in range(2):
                                  nc.vector.tensor_copy(
                                      v_sb[t][s][:, blk * 130 + hh * 65:blk * 130 + hh * 65 + 64],
                                      ptv[:, hh * 64:hh * 64 + 64])

                  def emit_attn(s):
                      for sl in range(4):
                          t, poff = sl // 2, (sl % 2) * 64
                          cap = dcaps[sl]
                          ncs = ncols[sl]
                          if causal:
                              j_first = max(0, 4 * s - cap + 1)
                              j_range = list(range(j_first, 4 * s + 4))
                          else:
                              j_first = 0
                              j_range = list(range(NB))
                          O_ps = opp.tile([65, 512], F32, tag="O")
                          for jj in j_range:
                              if causal and jj > j_first:
                                  q0 = max(512 * s, 128 * jj)
                                  qe = min(512 * (s + 1), 128 * (jj + cap))
                              else:
                                  q0, qe = 512 * s, 512 * (s + 1)
                              N = qe - q0
                              sp = spp.tile([128, 512], F32, tag="sp")
                              nc.tensor.matmul(
                                  sp[:, 0:N],
                                  k_sb[t][jj // 4][poff:poff + 64,
                                                   (jj % 4) * 128:(jj % 4) * 128 + 128],
                                  q_sb[t][s][poff:poff + 64, q0 - 512 * s:qe - 512 * s],
                                  start=True, stop=not use_dram_bias)
                              if use_dram_bias:
                                  bt_t = btp.tile([128, 512], F32R, tag="bt")
                                  nc.sync.dma_start(
                                      out=bt_t[:, 0:N],
                                      in_=biast[sl, jj * 128:(jj + 1) * 128, q0:qe])
                                  nc.tensor.matmul(sp[:, 0:N], ident_sb[:, :],
                                                   bt_t[:, 0:N], start=False, stop=True)
                              pt = ptp.tile([128, 512], F32R, tag="pt")
                              b_lo, b_hi = q0 // 128, qe // 128
                              for g in range(b_lo // ncs, (b_hi - 1) // ncs + 1):
                                  ba = max(b_lo, g * ncs)
                                  bb = min(b_hi, (g + 1) * ncs)
                                  u0 = ba * 128 - q0
                                  fd = (bb - ba) * 128
                                  dd = g * ncs - jj
                                  if use_dram_bias:
                                      bias = stab_sb[:, 0:1]
                                  else:
                                      col = sl * 20 + dd + 3
                                      bias = btab_sb[:, col:col + 1]
                                  nc.scalar.activation(pt[:, u0:u0 + fd],
                                                       sp[:, u0:u0 + fd],
                                                       AF.Exp, bias=bias, scale=1.0)
                              if causal and jj // 4 == s:
                                  o = 128 * jj - q0
                                  nc.gpsimd.affine_select(
                                      out=pt[:, o:o + 128], in_=pt[:, o:o + 128],
                                      compare_op=ALU.is_ge, fill=0.0,
                                      base=0, pattern=[[1, 128]],
                                      channel_multiplier=-1)
                              nc.tensor.matmul(
                                  O_ps[:, q0 - 512 * s:qe - 512 * s],
                                  v_sb[t][jj // 4][:, (jj % 4) * 130 + (sl % 2) * 65:
                                                   (jj % 4) * 130 + (sl % 2) * 65 + 65],
                                  pt[:, 0:N],
                                  start=(jj == j_first),
                                  stop=(jj == j_range[-1]))
                          # normalization for this (slot, supertile)
                          lr = lrp.tile([1, 512], F32R, tag="lr")
                          nc.vector.tensor_copy(lr[:, :], O_ps[64:65, :])
                          lbc = lbp.tile([64, 512], F32, tag="lbcop")
                          nc.tensor.matmul(lbc[:, :], ones_sb[:, :], lr[:, :],
                                           start=True, stop=True)
                          linv = lvp.tile([64, 512], F32, tag="linv")
                          nc.vector.reciprocal(linv[:, :], lbc[:, :])
                          nc.vector.tensor_tensor(
                              ao_sb[t][s][poff:poff + 64, :],
                              O_ps[0:64, :], linv[:, :], ALU.mult)
                  def emit_outproj(s):
                      for blk in range(4):
                          gb = s * 4 + blk
                          for half in range(2):
                              op_ps = lbp.tile([128, 512], F32, tag="lbcop")
                              for t in range(2):
                                  nc.tensor.matmul(
                                      op_ps[:, :],
                                      ao_sb[t][s][:, blk * 128:(blk + 1) * 128],
                                      wout_sb[:, t * 1024 + half * 512:
                                              t * 1024 + half * 512 + 512],
                                      start=(t == 0), stop=(t == 1))
                              os_t = osp.tile([128, 512], F32, tag="os")
                              nc.vector.tensor_copy(os_t[:, :], op_ps[:, :])
                              nc.sync.dma_start(
                                  out=out[gb * 128:(gb + 1) * 128,
                                          half * 512:half * 512 + 512],
                                  in_=os_t[:, :])


                  for s in range(NS):
                      emit_qkv(s)
                      emit_attn(s)
                      emit_outproj(s)

    nc.compile()
    return nc


def _get_program(variant, dcaps, ncols, stab, repeat=1):
    key = (variant, tuple(dcaps), tuple(ncols), float(stab), repeat)
    if key not in _cache:
        if variant == "fast2":
            _cache[key] = _build2(dcaps, ncols, repeat)
        else:
            _cache[key] = _build(variant, dcaps, ncols, stab, repeat)
    return _cache[key]


def _build2(dcaps, ncols, repeat=1):
    """Redesigned fast path: bf16 operands, direct token-major V (no PE
    transposes), tight ALiBi block caps, gpsimd partition-broadcast for the
    softmax normalization, per-supertile interleaved emission."""
    import concourse.bacc as bacc
    import concourse.mybir as mybir
    from concourse.tile import TileContext

    F32 = mybir.dt.float32
    F32R = mybir.dt.float32r
    BF16 = mybir.dt.bfloat16
    AF = mybir.ActivationFunctionType
    ALU = mybir.AluOpType

    nc = bacc.Bacc("TRN2", target_bir_lowering=False, debug=False)

    xtd = nc.dram_tensor("xtd", [128, 8 * L], BF16, kind="ExternalInput")
    wq = nc.dram_tensor("wq", [128, 2048], BF16, kind="ExternalInput")
    wk = nc.dram_tensor("wk", [128, 2048], BF16, kind="ExternalInput")
    wv = nc.dram_tensor("wv", [128, 2048], BF16, kind="ExternalInput")
    wout = nc.dram_tensor("wout", [128, 2048], BF16, kind="ExternalInput")
    btab = nc.dram_tensor("btab", [128, 80], F32, kind="ExternalInput")
    vonesd = nc.dram_tensor("vonesd", [128, 16], BF16, kind="ExternalInput")
    out = nc.dram_tensor("out", [L, D], BF16, kind="ExternalOutput")

    with TileContext(nc) as tc:
        with tc.tile_pool(name="const", bufs=1) as cp:
            wq_sb = cp.tile([128, 2048], BF16, tag="wq")
            wk_sb = cp.tile([128, 2048], BF16, tag="wk")
            wv_sb = cp.tile([128, 2048], BF16, tag="wv")
            wout_sb = cp.tile([128, 2048], BF16, tag="wout")
            btab_sb = cp.tile([128, 80], F32, tag="btab")
            nc.sync.dma_start(out=wq_sb[:, :], in_=wq[:, :])
            nc.sync.dma_start(out=wk_sb[:, :], in_=wk[:, :])
            nc.sync.dma_start(out=wv_sb[:, :], in_=wv[:, :])
            nc.sync.dma_start(out=wout_sb[:, :], in_=wout[:, :])
            nc.sync.dma_start(out=btab_sb[:, :], in_=btab[:, :])

            # q/k/v are parity-double-buffered (rep r uses set r%2) so the
            # next rep's QKV filler can write while this rep's attention
            # still reads the other set
            q_sb = [[[cp.tile([128, 512], BF16, tag=f"q{pr}t{t}s{s}",
                              name=f"q{pr}t{t}s{s}") for s in range(NS)]
                     for t in range(2)] for pr in range(2)]
            k_sb = [[[cp.tile([128, 512], BF16, tag=f"k{pr}t{t}s{s}",
                              name=f"k{pr}t{t}s{s}") for s in range(NS)]
                     for t in range(2)] for pr in range(2)]
            # v: token-major; per supertile 4 tok-blocks x (4 heads x 65)
            # (col 64 of each 65-group is the ones column for the l-row)
            v_sb = [[cp.tile([128, 1040], BF16, tag=f"v{pr}s{s}",
                             name=f"v{pr}s{s}") for s in range(NS)]
                    for pr in range(2)]
            ao_sb = [[cp.tile([128, 512], BF16, tag=f"ao{t}s{s}", name=f"ao{t}s{s}")
                      for s in range(NS)] for t in range(2)]

            vo3 = vonesd.rearrange("p (g c) -> p g c", c=1)
            for pr in range(2):
                for s in range(NS):
                    v4c = v_sb[pr][s].rearrange("p (g c) -> p g c", c=65)
                    nc.sync.dma_start(out=v4c[:, :, 64:65], in_=vo3[:, 0:16, :])

            with tc.tile_pool(name="xtp", bufs=2) as xtp, \
                 tc.tile_pool(name="qkp", bufs=1, space="PSUM") as qkp, \
                 tc.tile_pool(name="vps", bufs=1, space="PSUM") as vps, \
                 tc.tile_pool(name="spp", bufs=3, space="PSUM") as spp, \
                 tc.tile_pool(name="opp", bufs=2, space="PSUM") as opp, \
                 tc.tile_pool(name="opj", bufs=1, space="PSUM") as opj, \
                 tc.tile_pool(name="ptp", bufs=4) as ptp, \
                 tc.tile_pool(name="lvp", bufs=4) as lvp, \
                 tc.tile_pool(name="lbp", bufs=2) as lbp, \
                 tc.tile_pool(name="osp", bufs=3) as osp:

                def load_xt(rep):
                    xts = []
                    for d in range(8):
                        xt_t = xtp.tile([128, L], BF16, tag=f"xt{d}",
                                        name=f"xt_r{rep}d{d}")
                        nc.sync.dma_start(out=xt_t[:, :],
                                          in_=xtd[:, d * L:(d + 1) * L])
                        xts.append(xt_t)
                    return xts

                def emit_qk(xts, pr, s, t):
                    for wsb, dst in ((wq_sb, q_sb), (wk_sb, k_sb)):
                        p = qkp.tile([128, 512], F32, tag="qk", name="pqk")
                        for d in range(8):
                            nc.tensor.matmul(
                                p[:, :], wsb[:, d * 256 + t * 128:
                                             d * 256 + t * 128 + 128],
                                xts[d][:, s * 512:(s + 1) * 512],
                                start=(d == 0), stop=(d == 7))
                        nc.scalar.activation(dst[pr][t][s][:, :], p[:, :],
                                             AF.Identity)

                def emit_v(xts, pr, s, half):
                    # token-major v for tok blocks [2*half, 2*half+1]
                    for blk in (2 * half, 2 * half + 1):
                        pv = vps.tile([128, 256], F32, tag="v", name="pv")
                        for d in range(8):
                            nc.tensor.matmul(
                                pv[:, :],
                                xts[d][:, (s * 4 + blk) * 128:
                                       (s * 4 + blk) * 128 + 128],
                                wv_sb[:, d * 256:(d + 1) * 256],
                                start=(d == 0), stop=(d == 7))
                        v4 = v_sb[pr][s].rearrange("p (blk h c) -> p blk h c",
                                                   h=4, c=65)
                        nc.scalar.activation(v4[:, blk, :, 0:64],
                                             pv[:, :], AF.Identity)

                def emit_attn_slot(pr, s, sl):
                    t, poff = sl // 2, (sl % 2) * 64
                    cap = dcaps[sl]
                    ncs = ncols[sl]
                    j_first = max(0, 4 * s - cap + 1)
                    js = list(range(j_first, 4 * s + 4))
                    O = opp.tile([65, 512], F32, tag="O", name="O")
                    for jj in js:
                        q0 = max(512 * s, 128 * jj)
                        qe = min(512 * (s + 1), 128 * (jj + cap))
                        N = qe - q0
                        sp = spp.tile([128, 512], F32, tag="sp", name="sp")
                        nc.tensor.matmul(
                            sp[:, 0:N],
                            k_sb[pr][t][jj // 4][poff:poff + 64,
                                                 (jj % 4) * 128:
                                                 (jj % 4) * 128 + 128],
                            q_sb[pr][t][s][poff:poff + 64,
                                           q0 - 512 * s:qe - 512 * s],
                            start=True, stop=True)
                        pt = ptp.tile([128, 512], BF16, tag="pt", name="pt")
                        b_lo, b_hi = q0 // 128, qe // 128
                        for g in range(b_lo // ncs, (b_hi - 1) // ncs + 1):
                            ba = max(b_lo, g * ncs)
                            bb = min(b_hi, (g + 1) * ncs)
                            u0 = ba * 128 - q0
                            fd = (bb - ba) * 128
                            dd = g * ncs - jj
                            col = sl * 20 + dd + 3
                            nc.scalar.activation(pt[:, u0:u0 + fd],
                                                 sp[:, u0:u0 + fd],
                                                 AF.Exp,
                                                 bias=btab_sb[:, col:col + 1],
                                                 scale=1.0)
                        if jj >= 4 * s:
                            nc.gpsimd.affine_select(
                                out=pt[:, 0:128], in_=pt[:, 0:128],
                                compare_op=ALU.is_ge, fill=0.0,
                                base=0, pattern=[[1, 128]],
                                channel_multiplier=-1)
                        nc.tensor.matmul(
                            O[:, q0 - 512 * s:qe - 512 * s],
                            v_sb[pr][jj // 4][:, (jj % 4) * 260 + sl * 65:
                                              (jj % 4) * 260 + sl * 65 + 65],
                            pt[:, 0:N],
                            start=(jj == js[0]), stop=(jj == js[-1]))
                    linv = lvp.tile([1, 512], F32, tag="linv", name="linv")
                    nc.vector.reciprocal(linv[:, :], O[64:65, :])
                    lb = lbp.tile([64, 512], F32, tag="lb", name="lb")
                    nc.gpsimd.partition_broadcast(lb[:, :], linv[:, :])
                    nc.vector.tensor_tensor(
                        ao_sb[t][s][poff:poff + 64, :],
                        O[0:64, :], lb[:, :], ALU.mult)

                def emit_outproj(s):
                    for blk in range(4):
                        os_t = osp.tile([128, 1024], BF16, tag="os", name="os")
                        for half in range(2):
                            op = opj.tile([128, 512], F32, tag="op", name="op")
                            for t2 in range(2):
                                nc.tensor.matmul(
                                    op[:, :],
                                    ao_sb[t2][s][:, blk * 128:(blk + 1) * 128],
                                    wout_sb[:, t2 * 1024 + half * 512:
                                            t2 * 1024 + half * 512 + 512],
                                    start=(t2 == 0), stop=(t2 == 1))
                            nc.vector.tensor_copy(
                                os_t[:, half * 512:(half + 1) * 512],
                                op[:, :])
                        nc.sync.dma_start(
                            out=out[(s * 4 + blk) * 128:
                                    (s * 4 + blk + 1) * 128, :],
                            in_=os_t[:, :])

                # flat schedule over global supertiles with cross-rep
                # QKV interleaving (fillers keep the PE fed through every
                # rep boundary)
                xts_by_rep = {0: load_xt(0)}
                for t in range(2):
                    emit_qk(xts_by_rep[0], 0, 0, t)
                for half in range(2):
                    emit_v(xts_by_rep[0], 0, 0, half)
                NG = NS * repeat
                for gidx in range(NG):
                    rep, s = gidx // NS, gidx % NS
                    pr = rep % 2
                    if s == 1 and rep + 1 < repeat:
                        xts_by_rep[rep + 1] = load_xt(rep + 1)
                        xts_by_rep.pop(rep - 1, None)
                    for i, sl in enumerate((3, 2, 1, 0)):
                        emit_attn_slot(pr, s, sl)
                        g2 = gidx + 1
                        if g2 < NG:
                            rep2, s2 = g2 // NS, g2 % NS
                            xts2 = xts_by_rep[rep2]
                            if i < 2:
                                emit_qk(xts2, rep2 % 2, s2, i)
                            else:
                                emit_v(xts2, rep2 % 2, s2, i - 2)
                    emit_outproj(s)

    nc.compile()
    return nc


def _fast2_params():
    """dcaps/ncols for the fast2 path, derived from ALiBi decay."""
    slopes = (2.0 ** (-8.0 * (np.arange(1, H + 1) / H))).astype(np.float64)
    # keep key block jj for query block i iff i - jj < dcap;
    # validated numerically: per-head error < ~5e-4 with these caps
    dk = [max(2, int(np.ceil(8.0 / (128.0 * s))) + 1) for s in slopes]
    dcaps = [min(16, max(dk[4 * k + ci] for ci in range(4))) for k in range(4)]
    ncl = [max(1, min(4, int((72.0 / s + 64.0) // 128))) for s in slopes]
    ncols = [min(ncl[4 * k + ci] for ci in range(4)) for k in range(4)]
    return dcaps, ncols


def _per_core_inputs2(x, Wqkv, bqkv, Wout):
    import ml_dtypes
    BF = ml_dtypes.bfloat16
    x = np.asarray(x, dtype=np.float32)
    Wqkv = np.asarray(Wqkv, dtype=np.float32)
    Wout = np.asarray(Wout, dtype=np.float32)
    slopes = (2.0 ** (-8.0 * (np.arange(1, H + 1) / H))).astype(np.float64)

    # x^T d-tiled: xtd[p, d*L + c] = x[b][c, d*128 + p]
    xtds = []
    for b in range(B):
        xT = x[b].T.reshape(8, 128, L)          # [d, p, c]
        xtds.append(np.ascontiguousarray(
            xT.transpose(1, 0, 2).reshape(128, 8 * L)).astype(BF))

    def tile_w(w):  # [D, 256] -> [128, 8*256]
        return np.ascontiguousarray(
            w.reshape(8, 128, 256).transpose(1, 0, 2).reshape(128, 2048))

    p = np.arange(128, dtype=np.float64)
    vones = np.ones((128, 16), BF)
    maps = []
    for c in range(NCORES):
        bc, ci = c // 4, c % 4
        heads = [4 * k + ci for k in range(4)]
        qcols = np.concatenate([np.arange(h * 64, h * 64 + 64) for h in heads])
        wq_s = tile_w(Wqkv[:, qcols] * SCALE).astype(BF)
        wk_s = tile_w(Wqkv[:, D + qcols]).astype(BF)
        wv_s = tile_w(Wqkv[:, 2 * D + qcols]).astype(BF)

        btab_c = np.zeros((128, 80), np.float32)
        for k in range(4):
            s_h = slopes[heads[k]]
            for dd in range(-3, 17):
                btab_c[:, k * 20 + dd + 3] = (
                    -s_h * (128.0 * dd + 64.0 - p)).astype(np.float32)

        wout_rows = np.concatenate(
            [Wout[h * 64:h * 64 + 64, :] for h in heads], axis=0)  # [256, 1024]
        wout_c = np.ascontiguousarray(
            np.concatenate([wout_rows[0:128], wout_rows[128:256]],
                           axis=1)).astype(BF)

        maps.append({"xtd": xtds[bc], "wq": wq_s, "wk": wk_s, "wv": wv_s,
                     "wout": wout_c, "btab": btab_c, "vonesd": vones})
    return maps


def _detect_variant(alibi_bias, attn_mask):
    mask = np.asarray(attn_mask)
    tril = np.tril(np.ones((L, L), dtype=bool))
    mask_is_tril = mask.shape == (L, L) and np.array_equal(mask.astype(bool), tril)
    slopes = 2.0 ** (-8.0 * (np.arange(1, H + 1) / H))
    al = np.asarray(alibi_bias)
    alibi_ok = al.shape == (H, L, L)
    if alibi_ok:
        rel = (np.arange(L)[None, :] - np.arange(L)[:, None]).astype(np.float64)
        for h in range(H):
            exp_h = (slopes[h] * rel).astype(np.float32)
            if not np.allclose(al[h], exp_h, atol=1e-3, rtol=0.0):
                alibi_ok = False
                break
    if mask_is_tril and alibi_ok:
        return "fast"
    if mask_is_tril:
        return "causal_dram"
    return "full_dram"


def _per_core_inputs(x, alibi_bias, attn_mask, Wqkv, bqkv, Wout, variant):
    slopes = (2.0 ** (-8.0 * (np.arange(1, H + 1) / H))).astype(np.float64)
    # dkeep per head with +1 cushion: smallest d beyond which blocks underflow
    if variant == "fast":
        dk = [min(16, int(np.ceil(64.0 / (128.0 * s))) + 2) for s in slopes]
        dcaps = [max(dk[4 * k + ci] for ci in range(4)) for k in range(4)]
        ncl = [max(1, min(4, int((72.0 / s + 64.0) // 128))) for s in slopes]
        ncols = [min(ncl[4 * k + ci] for ci in range(4)) for k in range(4)]
    else:
        dcaps = [16, 16, 16, 16]
        ncols = [4, 4, 4, 4]

    x = np.asarray(x, dtype=np.float32)
    Wqkv = np.asarray(Wqkv, dtype=np.float32)
    bqkv = np.asarray(bqkv, dtype=np.float32)
    Wout = np.asarray(Wout, dtype=np.float32)

    xts = [np.ascontiguousarray(x[b].T) for b in range(B)]  # [D, L]

    stab = 0.0
    if variant != "fast":
        al = np.asarray(alibi_bias, dtype=np.float32)
        if variant == "causal_dram":
            mx = max(float(np.tril(al[h]).max()) for h in range(H))
        else:
            mk = np.asarray(attn_mask).astype(bool)
            mx = max(float(al[h][mk].max()) for h in range(H)) if mk.any() else 0.0
        stab = -(mx + 12.0)

    ident = np.eye(128, dtype=np.float32)
    onesd = np.ones((1, 64), np.float32)
    vones = np.ones((128, 16), np.float32)

    # fallback: transposed bias tensors, built once per ci (shared b0/b1 core)
    biast_by_ci = {}
    if variant != "fast":
        al = np.asarray(alibi_bias, dtype=np.float32)
        mb = None
        if variant == "full_dram":
            mb = np.where(np.asarray(attn_mask).astype(bool).T, 0.0,
                          np.float32(-1e30)).astype(np.float32)
        for ci in range(4):
            heads = [4 * k + ci for k in range(4)]
            bt = np.empty((4, L, L), np.float32)
            for k in range(4):
                bt[k] = al[heads[k]].T
                if mb is not None:
                    bt[k] += mb
            biast_by_ci[ci] = bt

    maps = []
    p = np.arange(128, dtype=np.float64)
    for c in range(NCORES):
        bc, ci = c // 4, c % 4
        heads = [4 * k + ci for k in range(4)]
        qcols = np.concatenate([np.arange(h * 64, h * 64 + 64) for h in heads])
        # weight slices [D, 256] -> tiled [128, 8*256]
        wq_s = Wqkv[:, qcols] * SCALE
        wk_s = Wqkv[:, D + qcols]
        wv_s = Wqkv[:, 2 * D + qcols]

        def tile_w(w):
            return np.ascontiguousarray(
                w.reshape(8, 128, 256).transpose(1, 0, 2).reshape(128, 2048))

        bq_s = (bqkv[qcols] * SCALE).reshape(2, 128).T.copy()    # [128, 2]
        bk_s = bqkv[D + qcols].reshape(2, 128).T.copy()

        btab_c = np.zeros((128, 80), np.float32)
        for k in range(4):
            s_h = slopes[heads[k]]
            for dd in range(-3, 16):
                btab_c[:, k * 20 + dd + 3] = (
                    -s_h * (128.0 * dd + 64.0 - p)).astype(np.float32)

        wout_rows = np.concatenate(
            [Wout[h * 64:h * 64 + 64, :] for h in heads], axis=0)  # [256, 1024]
        wout_c = np.ascontiguousarray(
            np.concatenate([wout_rows[0:128], wout_rows[128:256]], axis=1))

        m = {"xt": xts[bc], "wq": tile_w(wq_s), "wk": tile_w(wk_s),
             "wv": tile_w(wv_s), "bqv": bq_s, "bkv": bk_s, "btab": btab_c,
             "onesd": onesd, "vones": vones, "identd": ident, "wout": wout_c}
        if variant != "fast":
            m["biast"] = biast_by_ci[ci]
            m["stabd"] = np.full((128, 1), stab, np.float32)
        maps.append(m)
    return maps, dcaps, ncols, stab


def kernel(x, phylo_dists, alibi_bias, attn_mask, Wqkv, bqkv, Wout, bout,
           phylo_alpha):
    from concourse.bass_utils import run_bass_kernel_spmd

    variant = _detect_variant(alibi_bias, attn_mask)
    bq = np.asarray(bqkv, dtype=np.float32)
    if variant == "fast" and np.all(bq[:2 * D] == 0.0):
        variant = "fast2"

    if variant == "fast2":
        maps = _per_core_inputs2(x, Wqkv, bqkv, Wout)
        dcaps, ncols = _fast2_params()
        nc = _get_program("fast2", dcaps, ncols, 0.0)
    else:
        maps, dcaps, ncols, stab = _per_core_inputs(x, alibi_bias, attn_mask,
                                                    Wqkv, bqkv, Wout, variant)
        nc = _get_program(variant, dcaps, ncols, stab)
    res = run_bass_kernel_spmd(nc, maps, list(range(NCORES)))

    out = np.zeros((B, L, D), dtype=np.float32)
    for c in range(NCORES):
        out[c // 4] += np.asarray(res.results[c]["out"], dtype=np.float32)
    bv = bq[2 * D:3 * D]
    out += (bv @ np.asarray(Wout, dtype=np.float32) +
            np.asarray(bout, dtype=np.float32))[None, None, :]
    return out

